# revision 9
# baseline (speedup 1.0000x reference)
"""Trainium2 Bass kernel for the 3-layer GNN attention module.

Data-parallel over batch B=64 across 8 NeuronCores (8 batch elements each).
Per (batch, layer):
  Q/K = sigmoid(W@inp) via 0.5*tanh(0.5 z)+0.5 ; V computed transposed (Vt[m,r])
  St[m,n] = K^T Q  (scores, transposed layout: softmax axis m on partitions)
  Et = exp(inv_scale * St)  (bf16)
  rowsums (broadcast, doubled): ps_rs = twos^T @ Et   -> 2*sum_m Et[m,n] per column
  o[r,n]  = Vt^T @ Et (accumulated over m-blocks)
  recip2 = approx(1/ps_rs) = 0.5/rowsum ; rm = recip2 * mask (mask folded into silu arg)
  u' = (Wo@o) * rm = 0.5*u*mask ; silu(u)*mask = (tanh(u')+1)*u'
"""
import sys
sys.path.insert(0, "/opt/trn_rl_repo")
import numpy as np
import ml_dtypes

R, D, H, NLAYERS = 128, 64, 64, 3
B, N = 64, 1024
NCORES = 8
BPC = B // NCORES  # batches per core
NB = N // 128      # 8 m-blocks
BF16 = ml_dtypes.bfloat16

_compiled = {}
GPSIMD_RM = False
GROUP = 2
BUFS_QKV = 2
BUFS_MISC = 2
BUFS_INP = 2
BUFS_ET = 2


def _build_nc(reps: int = 1):
    import concourse.bass as bass
    from concourse import bacc, mybir
    from concourse.tile import TileContext
    from contextlib import ExitStack

    f32 = mybir.dt.float32
    bf16 = mybir.dt.bfloat16
    AF = mybir.ActivationFunctionType
    ALU = mybir.AluOpType

    nc = bacc.Bacc("TRN2", target_bir_lowering=False, debug=False, num_devices=NCORES)

    x_d = nc.dram_tensor("x", [BPC, D, N], bf16, kind="ExternalInput").ap()
    mask_d = nc.dram_tensor("mask", [BPC, N], bf16, kind="ExternalInput").ap()
    invsc_d = nc.dram_tensor("invsc", [128, BPC], f32, kind="ExternalInput").ap()
    w0_d = nc.dram_tensor("w0", [D, 3 * R], bf16, kind="ExternalInput").ap()
    wr_d = nc.dram_tensor("wr", [R, 2 * 3 * R], bf16, kind="ExternalInput").ap()
    wo_d = nc.dram_tensor("wo", [R, 2 * R], bf16, kind="ExternalInput").ap()
    wol_d = nc.dram_tensor("wol", [R, H], bf16, kind="ExternalInput").ap()
    out_d = nc.dram_tensor("out", [BPC, H, N], f32, kind="ExternalOutput").ap()

    with TileContext(nc) as tc, ExitStack() as ctx:
        singles = ctx.enter_context(tc.tile_pool(name="singles", bufs=1))
        pool_x = ctx.enter_context(tc.tile_pool(name="px", bufs=2))
        pool_inp = ctx.enter_context(tc.tile_pool(name="pinp", bufs=BUFS_INP))
        pool_qkv = ctx.enter_context(tc.tile_pool(name="pqkv", bufs=BUFS_QKV))
        pool_et = ctx.enter_context(tc.tile_pool(name="pet", bufs=BUFS_ET))
        pool_misc = ctx.enter_context(tc.tile_pool(name="pmisc", bufs=BUFS_MISC))
        pool_out = ctx.enter_context(tc.tile_pool(name="pout", bufs=2))
        pool_mm = ctx.enter_context(tc.tile_pool(name="pmm", bufs=2, space="PSUM"))
        pool_acc = ctx.enter_context(tc.tile_pool(name="pacc", bufs=1, space="PSUM"))

        # --- constants / weights (loaded once) ---
        w0_sb = singles.tile([D, 3 * R], bf16)
        nc.sync.dma_start(out=w0_sb, in_=w0_d)
        wr_sb = singles.tile([R, 2 * 3 * R], bf16)
        nc.sync.dma_start(out=wr_sb, in_=wr_d)
        wo_sb = singles.tile([R, 2 * R], bf16)
        nc.sync.dma_start(out=wo_sb, in_=wo_d)
        wol_sb = singles.tile([R, H], bf16)
        nc.sync.dma_start(out=wol_sb, in_=wol_d)
        invsc_sb = singles.tile([128, BPC], f32)
        nc.sync.dma_start(out=invsc_sb, in_=invsc_d)
        twos_sb = singles.tile([128, 128], bf16)
        nc.vector.memset(twos_sb, 2.0)
        # per-batch broadcast masks [128, N] each
        mask_sb = singles.tile([128, BPC, N], bf16)
        for b in range(BPC):
            nc.sync.dma_start(
                out=mask_sb[:, b, :], in_=mask_d[b][None, :].broadcast_to([128, N])
            )

        def layer_block(b, rin, l):
            Din = D if l == 0 else R
            if l == 0:
                wq_sl = w0_sb[:, 0:R]
                wk_sl = w0_sb[:, R:2 * R]
                wv_sl = w0_sb[:, 2 * R:3 * R]
            else:
                base = (l - 1) * 3 * R
                wq_sl = wr_sb[:, base:base + R]
                wk_sl = wr_sb[:, base + R:base + 2 * R]
                wv_sl = wr_sb[:, base + 2 * R:base + 3 * R]

            # --- Q = sigmoid(Wq @ rin), K likewise ---
            ps_q = pool_mm.tile([128, N], f32, tag="mm")
            for c in range(2):
                nc.tensor.matmul(ps_q[:, c * 512:(c + 1) * 512], lhsT=wq_sl,
                                 rhs=rin[:, c * 512:(c + 1) * 512],
                                 start=True, stop=True)
            Qt = pool_qkv.tile([128, N], bf16, tag=f"q{b % GROUP}")
            nc.scalar.activation(Qt, ps_q, AF.Tanh, scale=0.5)
            nc.vector.tensor_scalar(Qt, Qt, 0.5, 0.5, ALU.mult, ALU.add)

            ps_k = pool_mm.tile([128, N], f32, tag="mm")
            for c in range(2):
                nc.tensor.matmul(ps_k[:, c * 512:(c + 1) * 512], lhsT=wk_sl,
                                 rhs=rin[:, c * 512:(c + 1) * 512],
                                 start=True, stop=True)
            Kt = pool_qkv.tile([128, N], bf16, tag=f"k{b % GROUP}")
            nc.scalar.activation(Kt, ps_k, AF.Tanh, scale=0.5)
            nc.vector.tensor_scalar(Kt, Kt, 0.5, 0.5, ALU.mult, ALU.add)

            # --- Vt[m, r] ---
            ps_v = pool_mm.tile([128, N], f32, tag="mm")
            for j in range(NB):
                nc.tensor.matmul(ps_v[:, j * 128:(j + 1) * 128],
                                 lhsT=rin[:, j * 128:(j + 1) * 128],
                                 rhs=wv_sl, start=True, stop=True)
            Vt = pool_qkv.tile([128, N], bf16, tag=f"vt{b % GROUP}")
            nc.scalar.activation(Vt, ps_v, AF.Tanh, scale=0.5)
            nc.vector.tensor_scalar(Vt, Vt, 0.5, 0.5, ALU.mult, ALU.add)

            # --- attention ---
            Et = pool_et.tile([128, NB, N], bf16, tag=f"et{b % GROUP}")
            ps_rs = pool_acc.tile([128, N], f32, tag="rs")
            ps_o = pool_acc.tile([128, N], f32, tag="o")

            def st_exp(mb):
                ps_st = pool_mm.tile([128, N], f32, tag="mm")
                for c in range(2):
                    nc.tensor.matmul(ps_st[:, c * 512:(c + 1) * 512],
                                     lhsT=Kt[:, mb * 128:(mb + 1) * 128],
                                     rhs=Qt[:, c * 512:(c + 1) * 512],
                                     start=True, stop=True)
                nc.scalar.activation(Et[:, mb, :], ps_st, AF.Exp,
                                     scale=invsc_sb[:, b:b + 1])

            def rs_o(mb):
                for c in range(2):
                    nc.tensor.matmul(ps_rs[:, c * 512:(c + 1) * 512],
                                     lhsT=twos_sb,
                                     rhs=Et[:, mb, c * 512:(c + 1) * 512],
                                     start=(mb == 0), stop=(mb == NB - 1),
                                     skip_group_check=True)
                    nc.tensor.matmul(ps_o[:, c * 512:(c + 1) * 512],
                                     lhsT=Vt[:, mb * 128:(mb + 1) * 128],
                                     rhs=Et[:, mb, c * 512:(c + 1) * 512],
                                     start=(mb == 0), stop=(mb == NB - 1),
                                     skip_group_check=True)

            st_exp(0)
            for mb in range(1, NB):
                st_exp(mb)
                rs_o(mb - 1)
            rs_o(NB - 1)

            # --- normalize ---
            recip = pool_misc.tile([128, N], f32, tag=f"recip{b % GROUP}")
            nc.vector.reciprocal_approx_fast(recip, ps_rs)  # = 0.5/rowsum
            o_sb = pool_misc.tile([128, N], bf16, tag=f"osb{b % GROUP}")
            nc.vector.tensor_copy(o_sb, ps_o)
            return recip, o_sb

        def layer_fin(b, l, recip, o_sb):
            Hout = R if l < NLAYERS - 1 else H
            wo_sl = wo_sb[:, l * R:(l + 1) * R] if l < NLAYERS - 1 else wol_sb
            ps_t = pool_mm.tile([128, N], f32, tag="mm")
            for c in range(2):
                nc.tensor.matmul(ps_t[:Hout, c * 512:(c + 1) * 512], lhsT=wo_sl,
                                 rhs=o_sb[:, c * 512:(c + 1) * 512],
                                 start=True, stop=True)
            if l < NLAYERS - 1:
                rm = pool_misc.tile([128, N], f32, tag=f"rm{b % GROUP}")
                if GPSIMD_RM:
                    nc.gpsimd.tensor_mul(rm, recip, mask_sb[:, b, :])
                else:
                    nc.vector.tensor_mul(rm, recip, mask_sb[:, b, :])
            else:
                rm = recip
            up = pool_misc.tile([128, N], f32, tag=f"up{b % GROUP}")
            nc.vector.tensor_tensor(up[:Hout], ps_t[:Hout], rm[:Hout], ALU.mult)
            vt_ = pool_misc.tile([128, N], bf16, tag=f"v{b % GROUP}")
            nc.scalar.activation(vt_[:Hout], up[:Hout], AF.Tanh)
            if l < NLAYERS - 1:
                inp_t = pool_inp.tile([128, N], bf16, tag=f"inp{b % GROUP}")
                nc.vector.scalar_tensor_tensor(inp_t, vt_, 1.0, up,
                                               ALU.add, ALU.mult)
                return inp_t
            out_t = pool_out.tile([H, N], f32)
            nc.vector.scalar_tensor_tensor(out_t, vt_[:H], 1.0, up[:H],
                                           ALU.add, ALU.mult)
            nc.sync.dma_start(out=out_d[b], in_=out_t)
            return None

        for rep in range(reps):
            for g in range(BPC // GROUP):
                bs = [g * GROUP + i for i in range(GROUP)]
                rs = []
                for b in bs:
                    xt = pool_x.tile([D, N], bf16, tag=f"x{b % GROUP}")
                    nc.sync.dma_start(out=xt, in_=x_d[b])
                    rs.append(xt)
                for l in range(NLAYERS):
                    states = [layer_block(b, r, l) for b, r in zip(bs, rs)]
                    rs = [layer_fin(b, l, *s) for b, s in zip(bs, states)]
    nc.compile()
    return nc


def _get_nc():
    if "nc" not in _compiled:
        _compiled["nc"] = _build_nc()
    return _compiled["nc"]


def kernel(x, L, wq0, wqr, wk0, wkr, wv0, wvr, wor, wo_last):
    from concourse.bass_utils import run_bass_kernel_spmd

    x = np.asarray(x, np.float32)
    L = np.asarray(L)
    mask = L[:, 0, :].astype(np.float32)              # [B, N] in {0,1}
    num = mask.sum(axis=1) + 1.0
    invs = (1.0 / np.sqrt(num)).astype(np.float32)    # [B]

    wq0 = np.asarray(wq0, np.float32); wk0 = np.asarray(wk0, np.float32)
    wv0 = np.asarray(wv0, np.float32); wqr = np.asarray(wqr, np.float32)
    wkr = np.asarray(wkr, np.float32); wvr = np.asarray(wvr, np.float32)
    wor = np.asarray(wor, np.float32); wo_last = np.asarray(wo_last, np.float32)

    w0p = np.concatenate([wq0.T, wk0.T, wv0.T], axis=1).astype(BF16)       # [64, 384]
    wrp = np.concatenate(
        [np.concatenate([wqr[i].T, wkr[i].T, wvr[i].T], axis=1) for i in range(2)],
        axis=1).astype(BF16)                                               # [128, 768]
    wop = np.concatenate([wor[0].T, wor[1].T], axis=1).astype(BF16)        # [128, 256]
    wolp = wo_last.T.astype(BF16)                                          # [128, 64]

    nc = _get_nc()
    in_maps = []
    for c in range(NCORES):
        sl = slice(c * BPC, (c + 1) * BPC)
        in_maps.append({
            "x": x[sl].astype(BF16),
            "mask": mask[sl].astype(BF16),
            "invsc": np.ascontiguousarray(
                np.broadcast_to(invs[sl][None, :], (128, BPC))).astype(np.float32),
            "w0": w0p, "wr": wrp, "wo": wop, "wol": wolp,
        })
    res = run_bass_kernel_spmd(nc, in_maps, core_ids=list(range(NCORES)))
    out = np.concatenate([res.results[c]["out"] for c in range(NCORES)], axis=0)
    return out.astype(np.float32)


if __name__ == "__main__":
    nc = _build_nc()
    print("build+compile OK")


# revision 11
# speedup vs baseline: 605.6453x; 605.6453x over previous
"""Trainium2 Bass kernel for the 3-layer GNN attention module.

Data-parallel over batch B=64 across 8 NeuronCores (8 batch elements each).
Per (batch, layer):
  Q/K = sigmoid(W@inp) via 0.5*tanh(0.5 z)+0.5 ; V computed transposed (Vt[m,r])
  St[m,n] = K^T Q  (scores, transposed layout: softmax axis m on partitions)
  Et = exp(inv_scale * St)  (bf16)
  rowsums (broadcast, doubled): ps_rs = twos^T @ Et   -> 2*sum_m Et[m,n] per column
  o[r,n]  = Vt^T @ Et (accumulated over m-blocks)
  recip2 = approx(1/ps_rs) = 0.5/rowsum ; rm = recip2 * mask (mask folded into silu arg)
  u' = (Wo@o) * rm = 0.5*u*mask ; silu(u)*mask = (tanh(u')+1)*u'
"""
import sys
sys.path.insert(0, "/opt/trn_rl_repo")
import numpy as np
import ml_dtypes

R, D, H, NLAYERS = 128, 64, 64, 3
B, N = 64, 1024
NCORES = 8
BPC = B // NCORES  # batches per core
NB = N // 128      # 8 m-blocks
BF16 = ml_dtypes.bfloat16

_compiled = {}
GPSIMD_RM = False
GROUP = 4
BUFS_QKV = 1
BUFS_MISC = 1
BUFS_INP = 1
BUFS_ET = 1


def _build_nc(reps: int = 1):
    import concourse.bass as bass
    from concourse import bacc, mybir
    from concourse.tile import TileContext
    from contextlib import ExitStack

    f32 = mybir.dt.float32
    bf16 = mybir.dt.bfloat16
    AF = mybir.ActivationFunctionType
    ALU = mybir.AluOpType

    nc = bacc.Bacc("TRN2", target_bir_lowering=False, debug=False, num_devices=NCORES)

    x_d = nc.dram_tensor("x", [BPC, D, N], bf16, kind="ExternalInput").ap()
    mask_d = nc.dram_tensor("mask", [BPC, N], bf16, kind="ExternalInput").ap()
    invsc_d = nc.dram_tensor("invsc", [128, BPC], f32, kind="ExternalInput").ap()
    w0_d = nc.dram_tensor("w0", [D, 3 * R], bf16, kind="ExternalInput").ap()
    wr_d = nc.dram_tensor("wr", [R, 2 * 3 * R], bf16, kind="ExternalInput").ap()
    wo_d = nc.dram_tensor("wo", [R, 2 * R], bf16, kind="ExternalInput").ap()
    wol_d = nc.dram_tensor("wol", [R, H], bf16, kind="ExternalInput").ap()
    out_d = nc.dram_tensor("out", [BPC, H, N], f32, kind="ExternalOutput").ap()

    with TileContext(nc) as tc, ExitStack() as ctx:
        singles = ctx.enter_context(tc.tile_pool(name="singles", bufs=1))
        pool_x = ctx.enter_context(tc.tile_pool(name="px", bufs=2))
        pool_inp = ctx.enter_context(tc.tile_pool(name="pinp", bufs=BUFS_INP))
        pool_qkv = ctx.enter_context(tc.tile_pool(name="pqkv", bufs=BUFS_QKV))
        pool_et = ctx.enter_context(tc.tile_pool(name="pet", bufs=BUFS_ET))
        pool_misc = ctx.enter_context(tc.tile_pool(name="pmisc", bufs=BUFS_MISC))
        pool_out = ctx.enter_context(tc.tile_pool(name="pout", bufs=2))
        pool_mm = ctx.enter_context(tc.tile_pool(name="pmm", bufs=2, space="PSUM"))
        pool_acc = ctx.enter_context(tc.tile_pool(name="pacc", bufs=1, space="PSUM"))

        # --- constants / weights (loaded once) ---
        w0_sb = singles.tile([D, 3 * R], bf16)
        nc.sync.dma_start(out=w0_sb, in_=w0_d)
        wr_sb = singles.tile([R, 2 * 3 * R], bf16)
        nc.sync.dma_start(out=wr_sb, in_=wr_d)
        wo_sb = singles.tile([R, 2 * R], bf16)
        nc.sync.dma_start(out=wo_sb, in_=wo_d)
        wol_sb = singles.tile([R, H], bf16)
        nc.sync.dma_start(out=wol_sb, in_=wol_d)
        invsc_sb = singles.tile([128, BPC], f32)
        nc.sync.dma_start(out=invsc_sb, in_=invsc_d)
        twos_sb = singles.tile([128, 128], bf16)
        nc.vector.memset(twos_sb, 2.0)
        # per-batch broadcast masks [128, N] each
        mask_sb = singles.tile([128, BPC, N], bf16)
        for b in range(BPC):
            nc.sync.dma_start(
                out=mask_sb[:, b, :], in_=mask_d[b][None, :].broadcast_to([128, N])
            )

        def layer_block(b, rin, l):
            Din = D if l == 0 else R
            if l == 0:
                wq_sl = w0_sb[:, 0:R]
                wk_sl = w0_sb[:, R:2 * R]
                wv_sl = w0_sb[:, 2 * R:3 * R]
            else:
                base = (l - 1) * 3 * R
                wq_sl = wr_sb[:, base:base + R]
                wk_sl = wr_sb[:, base + R:base + 2 * R]
                wv_sl = wr_sb[:, base + 2 * R:base + 3 * R]

            # --- Q = sigmoid(Wq @ rin), K likewise ---
            ps_q = pool_mm.tile([128, N], f32, tag="mm")
            for c in range(2):
                nc.tensor.matmul(ps_q[:, c * 512:(c + 1) * 512], lhsT=wq_sl,
                                 rhs=rin[:, c * 512:(c + 1) * 512],
                                 start=True, stop=True)
            Qt = pool_qkv.tile([128, N], bf16, tag=f"q{b % GROUP}")
            nc.scalar.activation(Qt, ps_q, AF.Tanh, scale=0.5)
            nc.vector.tensor_scalar(Qt, Qt, 0.5, 0.5, ALU.mult, ALU.add)

            ps_k = pool_mm.tile([128, N], f32, tag="mm")
            for c in range(2):
                nc.tensor.matmul(ps_k[:, c * 512:(c + 1) * 512], lhsT=wk_sl,
                                 rhs=rin[:, c * 512:(c + 1) * 512],
                                 start=True, stop=True)
            Kt = pool_qkv.tile([128, N], bf16, tag=f"k{b % GROUP}")
            nc.scalar.activation(Kt, ps_k, AF.Tanh, scale=0.5)
            nc.vector.tensor_scalar(Kt, Kt, 0.5, 0.5, ALU.mult, ALU.add)

            # --- Vt[m, r] ---
            ps_v = pool_mm.tile([128, N], f32, tag="mm")
            for j in range(NB):
                nc.tensor.matmul(ps_v[:, j * 128:(j + 1) * 128],
                                 lhsT=rin[:, j * 128:(j + 1) * 128],
                                 rhs=wv_sl, start=True, stop=True)
            Vt = pool_qkv.tile([128, N], bf16, tag=f"vt{b % GROUP}")
            nc.scalar.activation(Vt, ps_v, AF.Tanh, scale=0.5)
            nc.vector.tensor_scalar(Vt, Vt, 0.5, 0.5, ALU.mult, ALU.add)

            # --- attention ---
            Et = pool_et.tile([128, NB, N], bf16, tag=f"et{b % GROUP}")
            ps_rs = pool_acc.tile([128, N], f32, tag="rs")
            ps_o = pool_acc.tile([128, N], f32, tag="o")

            def st_exp(mb):
                ps_st = pool_mm.tile([128, N], f32, tag="mm")
                for c in range(2):
                    nc.tensor.matmul(ps_st[:, c * 512:(c + 1) * 512],
                                     lhsT=Kt[:, mb * 128:(mb + 1) * 128],
                                     rhs=Qt[:, c * 512:(c + 1) * 512],
                                     start=True, stop=True)
                nc.scalar.activation(Et[:, mb, :], ps_st, AF.Exp,
                                     scale=invsc_sb[:, b:b + 1])

            def rs_o(mb):
                for c in range(2):
                    nc.tensor.matmul(ps_rs[:, c * 512:(c + 1) * 512],
                                     lhsT=twos_sb,
                                     rhs=Et[:, mb, c * 512:(c + 1) * 512],
                                     start=(mb == 0), stop=(mb == NB - 1),
                                     skip_group_check=True)
                    nc.tensor.matmul(ps_o[:, c * 512:(c + 1) * 512],
                                     lhsT=Vt[:, mb * 128:(mb + 1) * 128],
                                     rhs=Et[:, mb, c * 512:(c + 1) * 512],
                                     start=(mb == 0), stop=(mb == NB - 1),
                                     skip_group_check=True)

            st_exp(0)
            for mb in range(1, NB):
                st_exp(mb)
                rs_o(mb - 1)
            rs_o(NB - 1)

            # --- normalize ---
            recip = pool_misc.tile([128, N], f32, tag=f"recip{b % GROUP}")
            nc.vector.reciprocal_approx_fast(recip, ps_rs)  # = 0.5/rowsum
            o_sb = pool_misc.tile([128, N], bf16, tag=f"osb{b % GROUP}")
            nc.vector.tensor_copy(o_sb, ps_o)
            return recip, o_sb

        def layer_fin(b, l, recip, o_sb):
            Hout = R if l < NLAYERS - 1 else H
            wo_sl = wo_sb[:, l * R:(l + 1) * R] if l < NLAYERS - 1 else wol_sb
            ps_t = pool_mm.tile([128, N], f32, tag="mm")
            for c in range(2):
                nc.tensor.matmul(ps_t[:Hout, c * 512:(c + 1) * 512], lhsT=wo_sl,
                                 rhs=o_sb[:, c * 512:(c + 1) * 512],
                                 start=True, stop=True)
            if l < NLAYERS - 1:
                rm = pool_misc.tile([128, N], f32, tag=f"rm{b % GROUP}")
                if GPSIMD_RM:
                    nc.gpsimd.tensor_mul(rm, recip, mask_sb[:, b, :])
                else:
                    nc.vector.tensor_mul(rm, recip, mask_sb[:, b, :])
            else:
                rm = recip
            up = pool_misc.tile([128, N], f32, tag=f"up{b % GROUP}")
            nc.vector.tensor_tensor(up[:Hout], ps_t[:Hout], rm[:Hout], ALU.mult)
            vt_ = pool_misc.tile([128, N], bf16, tag=f"v{b % GROUP}")
            nc.scalar.activation(vt_[:Hout], up[:Hout], AF.Tanh)
            if l < NLAYERS - 1:
                inp_t = pool_inp.tile([128, N], bf16, tag=f"inp{b % GROUP}")
                nc.vector.scalar_tensor_tensor(inp_t, vt_, 1.0, up,
                                               ALU.add, ALU.mult)
                return inp_t
            out_t = pool_out.tile([H, N], f32)
            nc.vector.scalar_tensor_tensor(out_t, vt_[:H], 1.0, up[:H],
                                           ALU.add, ALU.mult)
            nc.sync.dma_start(out=out_d[b], in_=out_t)
            return None

        for rep in range(reps):
            for g in range(BPC // GROUP):
                bs = [g * GROUP + i for i in range(GROUP)]
                rs = []
                for b in bs:
                    xt = pool_x.tile([D, N], bf16, tag=f"x{b % GROUP}")
                    nc.sync.dma_start(out=xt, in_=x_d[b])
                    rs.append(xt)
                for l in range(NLAYERS):
                    states = [layer_block(b, r, l) for b, r in zip(bs, rs)]
                    rs = [layer_fin(b, l, *s) for b, s in zip(bs, states)]
    nc.compile()
    return nc


def _get_nc():
    if "nc" not in _compiled:
        _compiled["nc"] = _build_nc()
    return _compiled["nc"]


def kernel(x, L, wq0, wqr, wk0, wkr, wv0, wvr, wor, wo_last):
    from concourse.bass_utils import run_bass_kernel_spmd

    x = np.asarray(x, np.float32)
    L = np.asarray(L)
    mask = L[:, 0, :].astype(np.float32)              # [B, N] in {0,1}
    num = mask.sum(axis=1) + 1.0
    invs = (1.0 / np.sqrt(num)).astype(np.float32)    # [B]

    wq0 = np.asarray(wq0, np.float32); wk0 = np.asarray(wk0, np.float32)
    wv0 = np.asarray(wv0, np.float32); wqr = np.asarray(wqr, np.float32)
    wkr = np.asarray(wkr, np.float32); wvr = np.asarray(wvr, np.float32)
    wor = np.asarray(wor, np.float32); wo_last = np.asarray(wo_last, np.float32)

    w0p = np.concatenate([wq0.T, wk0.T, wv0.T], axis=1).astype(BF16)       # [64, 384]
    wrp = np.concatenate(
        [np.concatenate([wqr[i].T, wkr[i].T, wvr[i].T], axis=1) for i in range(2)],
        axis=1).astype(BF16)                                               # [128, 768]
    wop = np.concatenate([wor[0].T, wor[1].T], axis=1).astype(BF16)        # [128, 256]
    wolp = wo_last.T.astype(BF16)                                          # [128, 64]

    nc = _get_nc()
    in_maps = []
    for c in range(NCORES):
        sl = slice(c * BPC, (c + 1) * BPC)
        in_maps.append({
            "x": x[sl].astype(BF16),
            "mask": mask[sl].astype(BF16),
            "invsc": np.ascontiguousarray(
                np.broadcast_to(invs[sl][None, :], (128, BPC))).astype(np.float32),
            "w0": w0p, "wr": wrp, "wo": wop, "wol": wolp,
        })
    res = run_bass_kernel_spmd(nc, in_maps, core_ids=list(range(NCORES)))
    out = np.concatenate([res.results[c]["out"] for c in range(NCORES)], axis=0)
    return out.astype(np.float32)


if __name__ == "__main__":
    nc = _build_nc()
    print("build+compile OK")
